# revision 4
# baseline (speedup 1.0000x reference)
"""Trainium2 Bass kernel v3 for the 3-head GCN block.

Computes, for x:(N,C,T,V)=(128,64,128,25), A:(3,25,25), Wd:(3,64,64):
    out = relu(BN(sum_h Wd_h @ (A_h-mix along V of x)) + x)

Changes vs baseline (aimed at the measured bottlenecks: Scalar 88%,
PE 74% busy, DMA only 50%):
  * bf16 end-to-end: x cast to bf16 on HOST, out returned as bf16 and
    upcast on host -> DMA volume halves (26.2MB -> 13.1MB per core).
  * 3 conv groups instead of 4: the residual no longer rides the PE.
    BN scale is folded into the conv weights (per-o columns), so the
    epilogue is a single fused DVE scalar_tensor_tensor:
        out_pre = (g_psum + shift[c]) + x          (one PSUM pass)
    followed by one fat per-pair Scalar relu (352cyc overhead amortized
    over 3200 elements).
  * Graph-stage PSUM dsts are 128-col-aligned slots, 4 chunks per bank,
    12 accumulating matmuls per bank -> 7 epilogue ops per pair not 32.
  * Mid-copies (3x data now) fused 3 chunks per instruction and split
    DVE/Scalar to balance engine time.
  * tsz=5 (25 full chunks + 1 of t=3) -> 26 chunks/pair instead of 32.
"""

import numpy as np
import ml_dtypes

import concourse.bass as bass
import concourse.tile as tile
from concourse import bacc, mybir
from concourse import bass_utils

BN_EPS = 1e-5

N, C, T, V, H = 128, 64, 128, 25, 3
NCORES = 8
NS = N // NCORES  # 16 batches per core
NPAIRS = NS // 2  # 8 two-batch tiles per core

TSZ = 5
CHUNKS = [(i * TSZ, TSZ) for i in range(T // TSZ)] + (
    [(T - T % TSZ, T % TSZ)] if T % TSZ else []
)
NCHUNK = len(CHUNKS)  # 26
GROUP = 4  # chunks per graph-psum bank (epilogue granularity)
COPYG = 2  # chunks per fused mid-copy
NCONV = H * 2 * 64  # 384 conv output columns

# which fused-copy groups go to DVE (rest to ScalarE); tuned for balance:
# DVE also does all STT epilogues, Scalar does the per-pair relu.
DVE_COPY_GROUPS = frozenset({0, 6, 12})

_CACHE = {}


def _build_nc():
    f32 = mybir.dt.float32
    bf16 = mybir.dt.bfloat16
    f8e3 = mybir.dt.float8e3
    add = mybir.AluOpType.add
    relu = mybir.ActivationFunctionType.Relu

    nc = bacc.Bacc("TRN2", target_bir_lowering=False, debug=False)

    x_d = nc.dram_tensor("x", (NS, C, T, V), bf16, kind="ExternalInput").ap()
    wdt_d = nc.dram_tensor("wdt", (128, H, 2, 64), bf16, kind="ExternalInput").ap()
    bd_d = nc.dram_tensor("bd", (TSZ * V, H, TSZ * V), bf16, kind="ExternalInput").ap()
    sh_d = nc.dram_tensor("sh", (128, 1), f32, kind="ExternalInput").ap()
    out_d = nc.dram_tensor("out", (NS, C, T, V), bf16, kind="ExternalOutput").ap()

    with tile.TileContext(nc) as tc:
        with (
            tc.tile_pool(name="consts", bufs=1) as consts,
            tc.tile_pool(name="xo", bufs=2) as xo,
            tc.tile_pool(name="zt", bufs=6) as ztp,
            tc.tile_pool(name="ps_zt", bufs=3, space="PSUM") as ps_zt,
            tc.tile_pool(name="ps_g", bufs=2, space="PSUM") as ps_g,
        ):
            # dispatch the first x load before the (tiny) consts so the
            # first conv can start as early as possible
            x_tiles0 = xo.tile([128, T, V], bf16, tag="x", bufs=3, name="x_tiles0")
            nc.sync.dma_start(
                out=x_tiles0[:],
                in_=x_d[0:2].rearrange("a c t v -> (a c) t v"),
            )
            wdt_sb = consts.tile([128, H, 2, 64], bf16)
            nc.sync.dma_start(out=wdt_sb[:], in_=wdt_d[:])
            bd_sb = consts.tile([TSZ * V, H, TSZ * V], bf16)
            nc.sync.dma_start(out=bd_sb[:], in_=bd_d[:])
            sh_sb = consts.tile([128, 1], f32)
            nc.sync.dma_start(out=sh_sb[:], in_=sh_d[:])

            # HAM warmup: keep the PE busy ~3.5us while the first x tile
            # streams in, so real matmuls start at 2.4GHz (K=8/8).
            warm_w = consts.tile([128, 128], bf16)
            nc.vector.memset(warm_w[:], 0.0)
            warm_ps = ps_g.tile([128, GROUP, 128], f32, tag="g_ps", name="warm_ps")
            for wi in range(48):
                nc.tensor.matmul(
                    warm_ps[:, 0, :],
                    lhsT=warm_w[:],
                    rhs=warm_w[:],
                    start=True,
                    stop=True,
                )

            deferred = [None]
            prev = [None]  # (flush_fn, cgi, zt_ps, cg_chunks) across pairs

            def emit_tail():
                if deferred[0] is not None:
                    pp, opre, otile = deferred[0]
                    th = T // 2
                    for h0 in (0, th):
                        nc.vector.tensor_scalar_max(
                            otile[:, h0 : h0 + th, :], opre[:, h0 : h0 + th, :], 0.0
                        )
                        nc.gpsimd.dma_start(
                            out=out_d[
                                2 * pp : 2 * pp + 2, :, h0 : h0 + th, :
                            ].rearrange("a c t v -> (a c) t v"),
                            in_=otile[:, h0 : h0 + th, :],
                        )
                    deferred[0] = None

            for p in range(NPAIRS):
                if p == 0:
                    x_tile = x_tiles0
                else:
                    x_tile = xo.tile([128, T, V], bf16, tag="x", bufs=3)
                    nc.sync.dma_start(
                        out=x_tile[:],
                        in_=x_d[2 * p : 2 * p + 2].rearrange("a c t v -> (a c) t v"),
                    )
                out_pre = xo.tile([128, T, V], bf16, tag="opre")
                out_tile = xo.tile([128, T, V], bf16, tag="o")

                # eg -> [g_ps tile, mms_emitted, mms_total]
                g_state = {}

                def emit_stt(
                    eg,
                    g_state=g_state,
                    out_pre=out_pre,
                    x_tile=x_tile,
                    out_tile=out_tile,
                    inline_tail=(p == NPAIRS - 1),
                    pp=p,
                ):
                    """Fused epilogue for epilogue-group eg: (g+shift)+x."""
                    g_ps = g_state[eg][0]
                    eg0 = eg * GROUP
                    chunk_ids = list(range(eg0, min(eg0 + GROUP, NCHUNK)))
                    tg0 = CHUNKS[eg0][0]
                    tcount = sum(CHUNKS[cj][1] for cj in chunk_ids)
                    if all(CHUNKS[cj][1] == TSZ for cj in chunk_ids):
                        nc.vector.scalar_tensor_tensor(
                            out=out_pre[:, tg0 : tg0 + tcount, :],
                            in0=g_ps[:, : len(chunk_ids), : TSZ * V],
                            scalar=sh_sb[:],
                            in1=x_tile[:, tg0 : tg0 + tcount, :],
                            op0=add,
                            op1=add,
                        )
                    else:
                        for es2, cj in enumerate(chunk_ids):
                            tj, tc_ = CHUNKS[cj]
                            nc.vector.scalar_tensor_tensor(
                                out=out_pre[:, tj : tj + tc_, :],
                                in0=g_ps[:, es2, : tc_ * V],
                                scalar=sh_sb[:],
                                in1=x_tile[:, tj : tj + tc_, :],
                                op0=add,
                                op1=add,
                            )
                    if inline_tail:
                        nc.vector.tensor_scalar_max(
                            out_tile[:, tg0 : tg0 + tcount, :],
                            out_pre[:, tg0 : tg0 + tcount, :],
                            0.0,
                        )
                        nc.gpsimd.dma_start(
                            out=out_d[
                                2 * pp : 2 * pp + 2, :, tg0 : tg0 + tcount, :
                            ].rearrange("a c t v -> (a c) t v"),
                            in_=out_tile[:, tg0 : tg0 + tcount, :],
                        )

                def emit_graph(ci, zt_sb, cs, g_state=g_state, emit_stt=emit_stt):
                    t0, tcnt = CHUNKS[ci]
                    Mc = tcnt * V
                    eg = ci // GROUP
                    if eg not in g_state:
                        n_in = min(GROUP, NCHUNK - eg * GROUP)
                        g_tile = ps_g.tile(
                            [128, GROUP, 128], f32, tag="g_ps", name="g_tile"
                        )
                        g_state[eg] = [g_tile, 0, H * n_in]
                    ent = g_state[eg]
                    for g in range(H):
                        nc.tensor.matmul(
                            ent[0][:, ci % GROUP, :Mc],
                            lhsT=zt_sb[:Mc, cs, g],
                            rhs=bd_sb[:Mc, g, :Mc],
                            start=(ent[1] == 0),
                            stop=(ent[1] == ent[2] - 1),
                        )
                        ent[1] += 1
                    if ent[1] == ent[2]:
                        emit_stt(eg)

                def emit_copy(cg, zt_ps, chunk_ids):
                    """Mid-copy for the chunks of copy-group cg."""
                    zt_sb = ztp.tile([128, COPYG, H, 2, 64], bf16, tag="zt_sb")
                    use_dve = cg in DVE_COPY_GROUPS
                    if all(CHUNKS[cj][1] == TSZ for cj in chunk_ids):
                        src = zt_ps[:125, : len(chunk_ids), :NCONV]
                        dst = zt_sb[:125, : len(chunk_ids)]
                        if use_dve:
                            nc.vector.tensor_copy(dst, src)
                        else:
                            nc.scalar.copy(dst, src)
                    else:
                        # group contains the partial chunk: exact APs
                        for cj in chunk_ids:
                            Mj = CHUNKS[cj][1] * V
                            sj = cj % COPYG
                            if use_dve:
                                nc.vector.tensor_copy(
                                    zt_sb[:Mj, sj], zt_ps[:Mj, sj, :NCONV]
                                )
                            else:
                                nc.scalar.copy(
                                    zt_sb[:Mj, sj], zt_ps[:Mj, sj, :NCONV]
                                )
                    return zt_sb

                def emit_graphs_for(cg, zt_sb, chunk_ids, emit_graph=emit_graph):
                    for cj in chunk_ids:
                        emit_graph(cj, zt_sb, cj % COPYG)

                # emit conv of copy-group g+1 BEFORE copy/graph of group g
                # so the PE FIFO never head-of-line blocks on the copies;
                # the pipeline extends ACROSS pair boundaries.
                copy_groups = [
                    list(range(s0, min(s0 + COPYG, NCHUNK)))
                    for s0 in range(0, NCHUNK, COPYG)
                ]
                for cgi, cg_chunks in enumerate(copy_groups):
                    zt_ps = ps_zt.tile([128, COPYG, 512], f32, tag="zt_ps")
                    for ci in cg_chunks:
                        t0, tcnt = CHUNKS[ci]
                        Mc = tcnt * V
                        nc.tensor.matmul(
                            zt_ps[:Mc, ci % COPYG, :NCONV],
                            lhsT=x_tile[:, t0 : t0 + tcnt, :],
                            rhs=wdt_sb[:],
                            start=True,
                            stop=True,
                        )
                    # copy this group NOW (dependency-ready; frees its psum
                    # asap and starts the copy one queue-hop earlier) ...
                    zt_sb = emit_copy(cgi, zt_ps, cg_chunks)
                    # ... but defer its graph MMs one group so the PE FIFO
                    # always has the next convs ahead of copy-dependent work
                    if prev[0] is not None:
                        pg, pcgi, pzsb, pchunks = prev[0]
                        pg(pcgi, pzsb, pchunks)
                    if cgi == 1:
                        emit_tail()  # previous pair's relu + out-dma
                    prev[0] = (emit_graphs_for, cgi, zt_sb, cg_chunks)
                if p == NPAIRS - 1:
                    pg, pcgi, pzsb, pchunks = prev[0]
                    pg(pcgi, pzsb, pchunks)
                    prev[0] = None
                else:
                    deferred[0] = (p, out_pre, out_tile)



            emit_tail()

    nc.compile()
    return nc


def _get_nc():
    if "nc" not in _CACHE:
        _CACHE["nc"] = _build_nc()
    return _CACHE["nc"]


def _host_consts(A, Wd, bd, gamma, beta, run_mean, run_var):
    A = np.asarray(A, np.float32)
    Wd = np.asarray(Wd, np.float32)
    bd = np.asarray(bd, np.float32)
    gamma = np.asarray(gamma, np.float32)
    beta = np.asarray(beta, np.float32)
    run_mean = np.asarray(run_mean, np.float32)
    run_var = np.asarray(run_var, np.float32)

    scale = gamma / np.sqrt(run_var + BN_EPS)  # (64,)
    shift = (bd.sum(axis=0) - run_mean) * scale + beta  # (64,)

    # wdt[(a,c), h, a', o] = delta_{aa'} Wd[h, o, c] * scale[o]
    wdt = np.zeros((2, 64, H, 2, 64), np.float32)
    for h in range(H):
        w = Wd[h].T * scale[None, :]  # [c, o]
        wdt[0, :, h, 0, :] = w
        wdt[1, :, h, 1, :] = w
    wdt = wdt.reshape(128, H, 2, 64).astype(ml_dtypes.bfloat16)

    M = TSZ * V
    bdm = np.zeros((M, H, M), np.float32)
    for h in range(H):
        for i in range(TSZ):
            bdm[i * V : (i + 1) * V, h, i * V : (i + 1) * V] = A[h].T
    bdm = bdm.astype(ml_dtypes.bfloat16)

    sh2 = np.tile(shift, 2)[:, None].astype(np.float32)  # (128,1)
    return wdt, bdm, sh2


def _in_maps(x, A, Wd, bd, gamma, beta, run_mean, run_var):
    xb = np.ascontiguousarray(
        np.asarray(x, np.float32).astype(ml_dtypes.bfloat16)
    )
    wdt, bdm, sh2 = _host_consts(A, Wd, bd, gamma, beta, run_mean, run_var)
    return [
        {
            "x": xb[i * NS : (i + 1) * NS],
            "wdt": wdt,
            "bd": bdm,
            "sh": sh2,
        }
        for i in range(NCORES)
    ]


def kernel(x, A, Wd, bd, gamma, beta, run_mean, run_var, _trace=False):
    nc = _get_nc()
    in_maps = _in_maps(x, A, Wd, bd, gamma, beta, run_mean, run_var)
    res = bass_utils.run_bass_kernel_spmd(
        nc, in_maps, core_ids=list(range(NCORES)), trace=_trace
    )
    out = np.concatenate(
        [np.asarray(r["out"]).astype(np.float32) for r in res.results], axis=0
    )
    _CACHE["last_results"] = res
    return out
